# revision 32
# baseline (speedup 1.0000x reference)
"""Macro-F1 kernel for Trainium2, 8 NeuronCores.

Host-side counting sort groups rows by true class into class-pure 128-row
chunks (each class padded to a multiple of 128 with sentinel rows whose
argmax is exactly class 0).  The device then never needs y_true or a
one-hot build:

Per core, tiles of TK*128 rows laid out [128p, TK, 128c] (physical shard
row = 128*b + p*TK + k holds row p of logical chunk b+k):
  - DVE:  rowmax via tensor_reduce (X axis, one instr per tile)
  - anti[r,p] = (x[r,p] < rowmax[r]) in {0,1}, exact fp32 compare, split
    across engines per chunk: J_DVE chunks as one broadcast tensor_tensor
    on DVE, 1-2 chunks on GpSimd, the rest on ACT (Sign(rowmax - x)).
  - PE :  per chunk one matmul, stationary=anti [128,128] bf16,
          moving=ones [128,1] -> column sums into PSUM slot [:, G] of a
          [128,512] bank.  985 chunks fit in 2 PSUM banks; no mid-kernel
          eviction.
Host: counts[chunk,p] = 128 - colsum_anti; regroup chunk count vectors by
class, subtract the sentinel contributions, fp64 macro-F1 epilogue.
"""

import sys
import time

if "/opt/trn_rl_repo" not in sys.path:
    sys.path.insert(0, "/opt/trn_rl_repo")

import numpy as np

import concourse.bacc as bacc
import concourse.mybir as mybir
import concourse.tile as tile
from concourse import bass_utils

C = 128
NCORES = 8
TK = 32                  # chunks (of 128 rows) per steady-state tile
RAMP = (2, 4, 8, 16)     # small leading tiles so compute starts early
BIG = np.float16(60000)  # sentinel rows: [BIG, 0, ..., 0] -> argmax == 0
EPS = 1e-12

_CACHE = {}


def _tiles(M):
    out = []
    b = 0
    for tk in RAMP:
        if b + tk > M:
            break
        out.append((b, tk))
        b += tk
    while b < M:
        tk = min(TK, M - b)
        if tk < TK and tk >= 8:
            # split the remainder so the pipeline drains in smaller quanta
            h = (tk + 1) // 2
            out.append((b, h))
            out.append((b + h, tk - h))
            b += tk
        else:
            out.append((b, tk))
            b += tk
    return out


def _sched_all(tiles):
    """Per-tile chunk split -> (g on GpSimd, j on DVE, rest on ACT).

    Chunk layout within a tile: [0:g] GpSimd, [g:g+j] DVE, [g+j:] ACT.
    The final two tiles run entirely on DVE so ACT/GpSimd (whose queues
    lag) are not on the drain critical path.
    """
    n = len(tiles)
    out = []
    for ti, (b, tk) in enumerate(tiles):
        if ti >= n - 2:
            g, j = 0, tk
        elif tk >= TK:
            g = 2 if ti % 4 != 3 else 1
            j = 12 if ti % 4 == 2 else 13
        else:
            g = 1 if tk >= 8 else 0
            j = max(0, (tk - g) * 2 // 3)
        out.append((g, min(j, tk - g)))
    return out


def _build(M):
    f32 = mybir.dt.float32
    f16 = mybir.dt.float16
    Alu = mybir.AluOpType
    Act = mybir.ActivationFunctionType

    R = M * 128
    NB = -(-M // 512)    # psum banks used

    nc = bacc.Bacc("TRN2", target_bir_lowering=False, debug=False,
                   num_devices=NCORES)
    yp = nc.dram_tensor("yp", [R, C], f16, kind="ExternalInput")
    out = nc.dram_tensor("out", [NB, C, 512], f32, kind="ExternalOutput")

    with tile.TileContext(nc) as tc:
        with (
            tc.tile_pool(name="const", bufs=1) as cpool,
            tc.tile_pool(name="xin", bufs=8) as xpool,
            tc.tile_pool(name="anti", bufs=6) as apool,
            tc.tile_pool(name="antig", bufs=8) as gpool,
            tc.tile_pool(name="mtree", bufs=3) as mpool,
            tc.tile_pool(name="small", bufs=6) as spool,
            tc.tile_pool(name="psum", bufs=1, space="PSUM") as psum,
        ):
            ones = cpool.tile([128, 1], f16)
            nc.vector.memset(ones[:], 1.0)
            banks = [psum.tile([C, 512], f32, name=f"bank{b}",
                               tag=f"bank{b}") for b in range(NB)]

            next_evict = 0

            def evict(bi):
                w = min(512, M - bi * 512)
                sb = spool.tile([C, 512], f32, name=f"osb{bi}",
                                tag=f"osb{bi}")
                nc.scalar.copy(sb[:, 0:w], banks[bi][:, 0:w])
                nc.sync.dma_start(out.ap()[bi, :, 0:w], sb[:, 0:w])

            tiles = _tiles(M)
            scheds = _sched_all(tiles)
            for ti, (b, tk) in enumerate(tiles):
                x = xpool.tile([128, tk, C], f16, tag="x")
                nc.sync.dma_start(
                    x[:],
                    yp.ap()[b * 128 : (b + tk) * 128, :].rearrange(
                        "(p k) c -> p k c", k=tk
                    ),
                )
                g, j = scheds[ti]
                # Pairwise-max tree: fully-packed f16 tensor_tensor ops run
                # in DVE 2x mode (the monolithic tensor_reduce cannot), so
                # the row-max costs ~2.7us/tile instead of 4.3us.
                m1 = mpool.tile([128, tk, 64], f16, tag="m1")
                nc.vector.tensor_tensor(
                    m1[:], x[:, :, 0:64], x[:, :, 64:128], op=Alu.max
                )
                m2 = mpool.tile([128, tk, 32], f16, tag="m2")
                nc.vector.tensor_tensor(
                    m2[:], m1[:, :, 0:32], m1[:, :, 32:64], op=Alu.max
                )
                m3 = mpool.tile([128, tk, 16], f16, tag="m3")
                nc.vector.tensor_tensor(
                    m3[:], m2[:, :, 0:16], m2[:, :, 16:32], op=Alu.max
                )
                m4 = mpool.tile([128, tk, 8], f16, tag="m4")
                nc.vector.tensor_tensor(
                    m4[:], m3[:, :, 0:8], m3[:, :, 8:16], op=Alu.max
                )
                rmax = spool.tile([128, tk], f32, tag="rmax")
                nc.vector.tensor_reduce(
                    rmax[:], m4[:], axis=mybir.AxisListType.X, op=Alu.max
                )
                # GpSimd chunks go to their own anti tile: their ~3us/chunk
                # latency then never gates the main anti tile or its reuse.
                if g:
                    anti_g = gpool.tile([128, g, C], f16, tag="antig")
                    for k in range(g):
                        nc.gpsimd.tensor_scalar(
                            anti_g[:, k, :], x[:, k, :],
                            rmax[:, k : k + 1], None, op0=Alu.is_lt,
                        )
                anti = apool.tile([128, tk - g, C], f16, tag="anti")
                if j:
                    nc.vector.tensor_tensor(
                        anti[:, 0:j, :], x[:, g : g + j, :],
                        rmax[:, g : g + j, None].broadcast_to([128, j, C]),
                        op=Alu.is_lt,
                    )
                for k in range(j, tk - g):
                    nc.scalar.activation(
                        anti[:, k, :], x[:, g + k, :], Act.Sign,
                        bias=rmax[:, g + k : g + k + 1], scale=-1.0,
                    )
                # main-anti matmuls first: PE finishes its reads of the big
                # anti tile promptly and only then stalls on the laggy
                # GpSimd chunks, so anti-buffer reuse is never gated on GS.
                for k in list(range(g, tk)) + list(range(g)):
                    G = b + k
                    src_ap = anti_g[:, k, :] if k < g else anti[:, k - g, :]
                    nc.tensor.matmul(
                        banks[G // 512][:, (G % 512) : (G % 512) + 1],
                        src_ap, ones[:],
                        start=True, stop=True,
                    )
                # banks whose last chunk just finished -> evict overlapped
                while next_evict < NB and (
                    b + tk >= (next_evict + 1) * 512 or b + tk == M
                ):
                    evict(next_evict)
                    next_evict += 1

    nc.compile()
    return nc


def _get_nc(M):
    if M not in _CACHE:
        _CACHE[M] = _build(M)
    return _CACHE[M]


def _layout(y_true):
    """Class-sorted chunk layout. Returns (src, chunk_class, n_c, chunks_c, M).

    src[g*128 + p] = original row index of row p of logical chunk g
    (-1 for sentinel rows).  chunk_class[g] in [0,C) or C for all-sentinel
    dummy chunks.
    """
    yt = np.asarray(y_true).astype(np.int64).ravel()
    n_c = np.bincount(yt, minlength=C).astype(np.int64)
    chunks_c = (n_c + 127) // 128
    M_total = int(chunks_c.sum())
    M = -(-M_total // NCORES)
    total_chunks = M * NCORES

    order = np.argsort(yt, kind="stable").astype(np.int64)
    starts = np.zeros(C, np.int64)
    starts[1:] = np.cumsum(chunks_c)[:-1]
    src = np.full(total_chunks * 128, -1, np.int64)
    dst = np.concatenate(
        [starts[c] * 128 + np.arange(n_c[c]) for c in range(C)]
    )
    src[dst] = order

    chunk_class = np.full(total_chunks, C, np.int64)
    chunk_class[:M_total] = np.repeat(np.arange(C), chunks_c)
    return src, chunk_class, n_c, chunks_c, M


def _shards(y_pred, src, M):
    """Per-core physical shards in the device's [p, k] tile layout (f16)."""
    yp = np.asarray(y_pred).astype(np.float16)
    tiles = _tiles(M)
    shards = []
    for i in range(NCORES):
        sc = src[i * M * 128 : (i + 1) * M * 128]
        phys = np.empty(M * 128, np.int64)
        for (b, tk) in tiles:
            blk = sc[b * 128 : (b + tk) * 128].reshape(tk, 128)
            phys[b * 128 : (b + tk) * 128] = blk.T.ravel()
        mask = phys < 0
        shard = yp[np.where(mask, 0, phys)]
        if mask.any():
            shard[mask] = np.float16(0.0)
            shard[mask, 0] = BIG
        shards.append(np.ascontiguousarray(shard))
    return shards


def _run(y_pred, y_true, trace=False):
    src, chunk_class, n_c, chunks_c, M = _layout(y_true)
    nc = _get_nc(M)
    shards = _shards(y_pred, src, M)
    in_maps = [{"yp": s} for s in shards]
    res = None
    for attempt in range(3):
        try:
            res = bass_utils.run_bass_kernel_spmd(
                nc, in_maps, core_ids=list(range(NCORES)), trace=trace
            )
            break
        except Exception:
            if attempt == 2:
                raise
            time.sleep(2.0)

    NB = -(-M // 512)
    counts_all = []
    for r in res.results:
        o = r["out"].astype(np.float64)            # [NB, C, 512]
        cs = o.transpose(0, 2, 1).reshape(NB * 512, C)[:M]  # colsums [M, C]
        counts_all.append(128.0 - cs)
    counts_all = np.concatenate(counts_all, 0)     # [8M, C]
    cm = np.zeros((C + 1, C), np.float64)
    np.add.at(cm, chunk_class, counts_all)
    cm = cm[:C]
    cm[:, 0] -= (chunks_c * 128 - n_c)             # sentinel rows -> pred 0
    diag = np.diagonal(cm)
    precision = diag / (cm.sum(axis=1) + EPS)
    recall = diag / (cm.sum(axis=0) + EPS)
    f1 = 2.0 * precision * recall / (precision + recall + EPS)
    return np.float32(f1.mean()), res


def kernel(y_pred, y_true):
    out, _ = _run(y_pred, y_true, trace=False)
    return out
